# revision 10
# baseline (speedup 1.0000x reference)
"""Trainium2 Bass kernel for nn_Critic (LSTM critic over T=512 steps).

Sharding: pure data parallel. B=256 batch rows are split across 8 cores
(32 rows each); all weights are replicated. The sequential LSTM scan runs
locally per core.

Everything on-chip is kept "feature major" (hidden unit on the partition
axis, batch on the free axis) so the recurrence needs no transposes:
  h, c, hmax       : [128, 64]   col = 32*k + b   (unit u = 128*k + p)
  z (gates, PSUM)  : [128, 256]  blocks of 32 cols = [i0 i1 f0 f1 o0 o1 g0 g1]
  xT (input proj)  : [97, T*32]  col = 32*t + b, rows 0:32 act, 32:96 inp2,
                                 row 96 = ones (carries fused bias)

Per scan step: 24 accumulating matmuls (16 Ul tiles + 8 [Wl|bias] tiles),
then sigmoid/tanh on ACT and the cell update on DVE.

Reference quirks honored:
  * inp3 = elu(empty @ Woi + boi) = elu(boi) broadcast -> constant; its
    contribution inp3 @ Wl[96:160] is folded into the z bias.
  * osc_state and Woi (shape [0,64]) are unused.
  * only osc[..., :64] is ever read.
"""

import os
import sys

sys.path.insert(0, "/opt/trn_rl_repo")

from contextlib import ExitStack

import numpy as np

import concourse.bass as bass
import concourse.bacc as bacc
import concourse.mybir as mybir
import concourse.tile as tile
from concourse.masks import make_identity

FP32 = mybir.dt.float32
AF = mybir.ActivationFunctionType
ALU = mybir.AluOpType

# Problem dims
B_FULL, T_FULL, A = 256, 512, 32
DM, DR = 64, 128
U = 256                 # lstm units (== combine units)
OSC_HALF = 64
NCORES = 8
B = B_FULL // NCORES    # 32 batch rows per core
XROWS = A + OSC_HALF    # 96 feature rows of xT (plus a ones row)

# psum gate-block beta -> source 128-col chunk of [Ul | Wl | bias] matrices.
# z layout blocks: [i0 i1 f0 f1 o0 o1 g0 g1]; weight col order is [i f g o].
SRC_CHUNK = [0, 1, 2, 3, 6, 7, 4, 5]


def _elu(nc, pool, out_ap, y_ap, shape):
    """out = elu(y) = max(y, exp(min(y, 0)) - 1), exact.

    y_ap may live in PSUM or SBUF. 2 DVE ops + 1 ACT op.
    """
    m = pool.tile(shape, FP32, tag="elu_m")
    nc.vector.tensor_scalar_min(m, y_ap, 0.0)
    e = pool.tile(shape, FP32, tag="elu_e")
    nc.scalar.activation(e, m, AF.Exp)
    # out = (e + (-1)) max y
    nc.vector.scalar_tensor_tensor(out_ap, e, -1.0, y_ap, ALU.add, ALU.max)


def build_nc(T=T_FULL):
    """Build the SPMD Bass program for one core (batch shard of 32)."""
    # Bacc (not plain Bass): its compile() runs move_matmul_waits_to_ldweights
    # + generate_event_semaphores, which split multi-wait instructions that
    # walrus codegen rejects ("Too many sync wait commands").
    nc = bacc.Bacc("TRN2", target_bir_lowering=False, debug=False)

    # ---- external I/O (per-core shard shapes) ----
    d_action = nc.dram_tensor("action", [B, T, A], FP32, kind="ExternalInput").ap()
    d_osc = nc.dram_tensor("osc", [B, T, 2 * OSC_HALF], FP32, kind="ExternalInput").ap()
    d_motion = nc.dram_tensor("motion_state", [B, DM], FP32, kind="ExternalInput").ap()
    d_robot = nc.dram_tensor("robot_state", [B, DR], FP32, kind="ExternalInput").ap()
    d_mu = nc.dram_tensor("mu", [B, A], FP32, kind="ExternalInput").ap()
    d_mean = nc.dram_tensor("mean", [B, A], FP32, kind="ExternalInput").ap()
    d_Wm = nc.dram_tensor("Wm", [DM, U], FP32, kind="ExternalInput").ap()
    d_bm = nc.dram_tensor("bm", [U], FP32, kind="ExternalInput").ap()
    d_Wr = nc.dram_tensor("Wr", [DR, U], FP32, kind="ExternalInput").ap()
    d_br = nc.dram_tensor("br", [U], FP32, kind="ExternalInput").ap()
    d_Wc = nc.dram_tensor("Wc", [2 * U, U], FP32, kind="ExternalInput").ap()
    d_bc = nc.dram_tensor("bc", [U], FP32, kind="ExternalInput").ap()
    d_Wor = nc.dram_tensor("Wor", [OSC_HALF, OSC_HALF], FP32, kind="ExternalInput").ap()
    d_bor = nc.dram_tensor("bor", [OSC_HALF], FP32, kind="ExternalInput").ap()
    d_boi = nc.dram_tensor("boi", [OSC_HALF], FP32, kind="ExternalInput").ap()
    d_Wl = nc.dram_tensor("Wl", [A + 2 * OSC_HALF, 4 * U], FP32, kind="ExternalInput").ap()
    d_bl = nc.dram_tensor("bl", [4 * U], FP32, kind="ExternalInput").ap()
    d_Ul = nc.dram_tensor("Ul", [U, 4 * U], FP32, kind="ExternalInput").ap()
    d_Wo = nc.dram_tensor("Wo", [U, 1], FP32, kind="ExternalInput").ap()
    d_bo = nc.dram_tensor("bo", [1], FP32, kind="ExternalInput").ap()
    d_out = nc.dram_tensor("out", [B, 1], FP32, kind="ExternalOutput").ap()

    with tile.TileContext(nc) as tc, ExitStack() as ctx:
        _build_body(
            ctx, tc, T,
            d_action, d_osc, d_motion, d_robot, d_mu, d_mean,
            d_Wm, d_bm, d_Wr, d_br, d_Wc, d_bc, d_Wor, d_bor, d_boi,
            d_Wl, d_bl, d_Ul, d_Wo, d_bo, d_out,
        )
    nc.finalize()
    return nc


def _build_body(ctx, tc, T,
                d_action, d_osc, d_motion, d_robot, d_mu, d_mean,
                d_Wm, d_bm, d_Wr, d_br, d_Wc, d_bc, d_Wor, d_bor, d_boi,
                d_Wl, d_bl, d_Ul, d_Wo, d_bo, d_out):
    nc = tc.nc
    TCH = T // 128          # 128-step chunks per batch row

    consts = ctx.enter_context(tc.tile_pool(name="consts", bufs=1))
    weights = ctx.enter_context(tc.tile_pool(name="weights", bufs=1))
    state = ctx.enter_context(tc.tile_pool(name="state", bufs=1))
    stage = ctx.enter_context(tc.tile_pool(name="stage", bufs=3))
    ptrans = ctx.enter_context(tc.tile_pool(name="ptrans", bufs=2, space="PSUM"))
    pmm = ctx.enter_context(tc.tile_pool(name="pmm", bufs=2, space="PSUM"))
    scratch = ctx.enter_context(tc.tile_pool(name="scratch", bufs=3))

    ident = consts.tile([128, 128], FP32)
    make_identity(nc, ident)
    ones_r = consts.tile([1, B], FP32)
    nc.vector.memset(ones_r, 1.0)

    # ---------------- weights to SBUF ----------------
    # Ul tiles [128,128]: ulw[k][beta] = Ul[128k:128k+128, chunk SRC_CHUNK[beta]]
    ulw = [[weights.tile([128, 128], FP32, tag=f"ul_{k}_{b}", name=f"ul_{k}_{b}") for b in range(8)]
           for k in range(2)]
    for k in range(2):
        for beta in range(8):
            m = SRC_CHUNK[beta]
            nc.sync.dma_start(
                out=ulw[k][beta],
                in_=d_Ul[128 * k:128 * (k + 1), 128 * m:128 * (m + 1)])

    # Wl tiles [97,128]: rows 0:96 = Wl[0:96, chunk], row 96 = fused bias
    # xT row layout: 0:64 = inp2 features, 64:96 = action features, 96 = ones
    # (engine APs must start at partition 0/32/64/96; a 64-row write may not
    # start at partition 32, so inp2 goes first). Wl tile rows permuted to match.
    wlw = [weights.tile([XROWS + 1, 128], FP32, tag=f"wl_{b}", name=f"wl_{b}") for b in range(8)]
    for beta in range(8):
        m = SRC_CHUNK[beta]
        nc.sync.dma_start(out=wlw[beta][0:OSC_HALF, :],
                          in_=d_Wl[A:A + OSC_HALF, 128 * m:128 * (m + 1)])
        nc.sync.dma_start(out=wlw[beta][OSC_HALF:XROWS, :],
                          in_=d_Wl[0:A, 128 * m:128 * (m + 1)])

    # fused bias blEff = bl + elu(boi) @ Wl[96:160, :]
    boi_sb = scratch.tile([OSC_HALF, 1], FP32)
    nc.sync.dma_start(out=boi_sb, in_=d_boi.rearrange("(p one) -> p one", one=1))
    eboi = scratch.tile([OSC_HALF, 1], FP32)
    _elu(nc, scratch, eboi, boi_sb, [OSC_HALF, 1])
    wl_hi = scratch.tile([OSC_HALF, 4 * U], FP32)
    nc.sync.dma_start(out=wl_hi, in_=d_Wl[XROWS:XROWS + OSC_HALF, :])
    p_bl = pmm.tile([1, 4 * U], FP32, tag="p_bl", bufs=1)
    for half in range(2):
        nc.tensor.matmul(p_bl[:, 512 * half:512 * (half + 1)],
                         eboi, wl_hi[:, 512 * half:512 * (half + 1)],
                         start=True, stop=True)
    bl_sb = scratch.tile([1, 4 * U], FP32)
    nc.sync.dma_start(out=bl_sb, in_=d_bl.rearrange("(one n) -> one n", one=1))
    bleff = scratch.tile([1, 4 * U], FP32)
    nc.vector.tensor_add(bleff, p_bl, bl_sb)
    for beta in range(8):
        m = SRC_CHUNK[beta]
        nc.vector.tensor_copy(wlw[beta][XROWS:XROWS + 1, :],
                              bleff[:, 128 * m:128 * (m + 1)])

    # [Wor; bor] [65, 64]
    worb = weights.tile([OSC_HALF + 1, OSC_HALF], FP32)
    nc.sync.dma_start(out=worb[0:OSC_HALF, :], in_=d_Wor)
    nc.sync.dma_start(out=worb[OSC_HALF:OSC_HALF + 1, :],
                      in_=d_bor.rearrange("(one n) -> one n", one=1))

    # [Wm; bm] chunks [65, 128]
    wmb = [weights.tile([DM + 1, 128], FP32, tag=f"wm_{c}", name=f"wm_{c}") for c in range(2)]
    for c in range(2):
        nc.sync.dma_start(out=wmb[c][0:DM, :], in_=d_Wm[:, 128 * c:128 * (c + 1)])
        nc.sync.dma_start(out=wmb[c][DM:DM + 1, :],
                          in_=d_bm.rearrange("(one n) -> one n", one=1)[:, 128 * c:128 * (c + 1)])
    # Wr chunks [128,128] + br rows [1,128]
    wrb = [weights.tile([DR, 128], FP32, tag=f"wr_{c}", name=f"wr_{c}") for c in range(2)]
    brb = [weights.tile([1, 128], FP32, tag=f"br_{c}", name=f"br_{c}") for c in range(2)]
    for c in range(2):
        nc.sync.dma_start(out=wrb[c], in_=d_Wr[:, 128 * c:128 * (c + 1)])
        nc.sync.dma_start(out=brb[c],
                          in_=d_br.rearrange("(one n) -> one n", one=1)[:, 128 * c:128 * (c + 1)])
    # Wc chunks [128,128] (4 k-chunks x 2 m-chunks) + bc rows
    wcb = [[weights.tile([128, 128], FP32, tag=f"wc_{k}_{c}", name=f"wc_{k}_{c}") for c in range(2)]
           for k in range(4)]
    bcb = [weights.tile([1, 128], FP32, tag=f"bc_{c}", name=f"bc_{c}") for c in range(2)]
    for k in range(4):
        for c in range(2):
            nc.sync.dma_start(out=wcb[k][c],
                              in_=d_Wc[128 * k:128 * (k + 1), 128 * c:128 * (c + 1)])
    for c in range(2):
        nc.sync.dma_start(out=bcb[c],
                          in_=d_bc.rearrange("(one n) -> one n", one=1)[:, 128 * c:128 * (c + 1)])
    # Wo chunks [128,1], bo [1,1]
    wob = [weights.tile([128, 1], FP32, tag=f"wo_{c}", name=f"wo_{c}") for c in range(2)]
    for c in range(2):
        nc.sync.dma_start(out=wob[c], in_=d_Wo[128 * c:128 * (c + 1), :])
    bob = weights.tile([1, 1], FP32)
    nc.sync.dma_start(out=bob, in_=d_bo.rearrange("(one n) -> one n", one=1))

    # muT/meanT [32a, 32b] via PE transpose
    mu_sb = scratch.tile([B, A], FP32)
    mean_sb = scratch.tile([B, A], FP32)
    nc.sync.dma_start(out=mu_sb, in_=d_mu)
    nc.sync.dma_start(out=mean_sb, in_=d_mean)
    muT = consts.tile([A, B], FP32)
    meanT = consts.tile([A, B], FP32)
    for src, dst in ((mu_sb, muT), (mean_sb, meanT)):
        pt = ptrans.tile([A, B], FP32, tag="pt", name="pt_mu")
        nc.tensor.transpose(pt, src, ident[0:B, 0:B])
        nc.vector.tensor_copy(dst, pt)

    # ---------------- xT: [97, T*32] feature-major input projection ----------
    xT = state.tile([XROWS + 1, T * B], FP32)
    nc.vector.memset(xT[XROWS:XROWS + 1, :], 1.0)

    # action -> xT[0:32]: per (b, j): transpose [128t, 32] -> [32, 128t],
    # then out = psum * muT[:,b] + meanT[:,b] scattered to cols 32*t + b.
    for b in range(B):
        for j in range(TCH):
            a_tile = stage.tile([128, A], FP32, tag="a_in")
            nc.sync.dma_start(out=a_tile, in_=d_action[b, 128 * j:128 * (j + 1), :])
            pt = ptrans.tile([A, 128], FP32, tag="pt", name="pt_a")
            nc.tensor.transpose(pt, a_tile, ident)
            dst = xT[OSC_HALF:XROWS, :].rearrange("p (t b) -> p t b", b=B)[:, 128 * j:128 * (j + 1), b]
            nc.vector.tensor_scalar(dst, pt, muT[:, b:b + 1], meanT[:, b:b + 1],
                                    ALU.mult, ALU.add)

    # osc[...,:64] -> inp2 -> xT[32:96]: per b: 4 transposes into oscT_b
    # [65, 512], one matmul vs [Wor;bor], elu, scatter to cols 32*t + b.
    for b in range(B):
        oscT_b = stage.tile([OSC_HALF + 1, T], FP32, tag="oscT")
        nc.vector.memset(oscT_b[OSC_HALF:OSC_HALF + 1, :], 1.0)
        for j in range(TCH):
            o_tile = stage.tile([128, OSC_HALF], FP32, tag="o_in")
            nc.sync.dma_start(out=o_tile,
                              in_=d_osc[b, 128 * j:128 * (j + 1), 0:OSC_HALF])
            pt = ptrans.tile([OSC_HALF, 128], FP32, tag="pt", name="pt_o")
            nc.tensor.transpose(pt, o_tile, ident)
            nc.vector.tensor_copy(oscT_b[0:OSC_HALF, 128 * j:128 * (j + 1)], pt)
        n_mm = (T + 511) // 512
        pw = pmm.tile([OSC_HALF, T], FP32, tag="mm", name="pw")
        for q in range(n_mm):
            w = min(512, T - 512 * q)
            nc.tensor.matmul(pw[:, 512 * q:512 * q + w], worb,
                             oscT_b[:, 512 * q:512 * q + w], start=True, stop=True)
        dst = xT[0:OSC_HALF, :].rearrange("p (t b) -> p t b", b=B)[:, :, b]
        _elu(nc, scratch, dst, pw, [OSC_HALF, T])

    # ---------------- h0 = c0 ----------------
    motT = scratch.tile([DM + 1, B], FP32)
    pt = ptrans.tile([DM, B], FP32, tag="pt", name="pt_mot")
    mot_sb = scratch.tile([B, DM], FP32)
    nc.sync.dma_start(out=mot_sb, in_=d_motion)
    nc.tensor.transpose(pt, mot_sb, ident[0:B, 0:B])
    nc.vector.tensor_copy(motT[0:DM, :], pt)
    nc.vector.memset(motT[DM:DM + 1, :], 1.0)

    robT = scratch.tile([DR, B], FP32)
    pt = ptrans.tile([DR, B], FP32, tag="pt", name="pt_rob")
    rob_sb = scratch.tile([B, DR], FP32)
    nc.sync.dma_start(out=rob_sb, in_=d_robot)
    nc.tensor.transpose(pt, rob_sb, ident[0:B, 0:B])
    nc.vector.tensor_copy(robT, pt)

    # ms = elu(motion @ Wm + bm): psum [128, 64] (col = 32*c + b)
    p_ms = pmm.tile([128, 2 * B], FP32, tag="mm", name="p_ms")
    for c in range(2):
        nc.tensor.matmul(p_ms[:, B * c:B * (c + 1)], wmb[c], motT,
                         start=True, stop=True)
    msT = scratch.tile([128, 2 * B], FP32, tag="msT")
    _elu(nc, scratch, msT, p_ms, [128, 2 * B])

    p_rs = pmm.tile([128, 2 * B], FP32, tag="mm", name="p_rs")
    for c in range(2):
        sl = p_rs[:, B * c:B * (c + 1)]
        nc.tensor.matmul(sl, wrb[c], robT, start=True, stop=False)
        nc.tensor.matmul(sl, brb[c], ones_r, start=False, stop=True)
    rsT = scratch.tile([128, 2 * B], FP32, tag="rsT")
    _elu(nc, scratch, rsT, p_rs, [128, 2 * B])

    p_st = pmm.tile([128, 2 * B], FP32, tag="mm", name="p_st")
    for c in range(2):
        sl = p_st[:, B * c:B * (c + 1)]
        nc.tensor.matmul(sl, wcb[0][c], msT[:, 0:B], start=True, stop=False)
        nc.tensor.matmul(sl, wcb[1][c], msT[:, B:2 * B], start=False, stop=False)
        nc.tensor.matmul(sl, wcb[2][c], rsT[:, 0:B], start=False, stop=False)
        nc.tensor.matmul(sl, wcb[3][c], rsT[:, B:2 * B], start=False, stop=False)
        nc.tensor.matmul(sl, bcb[c], ones_r, start=False, stop=True)

    h = state.tile([128, 2 * B], FP32)
    c_st = state.tile([128, 2 * B], FP32)
    hmax = state.tile([128, 2 * B], FP32)
    _elu(nc, scratch, h, p_st, [128, 2 * B])
    nc.vector.tensor_copy(c_st, h)
    nc.vector.memset(hmax, -1e30)

    # ---------------- the scan ----------------
    gates = ctx.enter_context(tc.tile_pool(name="gates", bufs=2))
    pz_pool = ctx.enter_context(tc.tile_pool(name="pz", bufs=2, space="PSUM"))
    BETA_ORDER = [6, 7, 0, 1, 2, 3, 4, 5]   # g first, o last
    for t in range(T):
        pz = pz_pool.tile([128, 256], FP32)
        xs = xT[:, B * t:B * (t + 1)]
        for beta in BETA_ORDER:
            sl = pz[:, 32 * beta:32 * (beta + 1)]
            nc.tensor.matmul(sl, ulw[0][beta], h[:, 0:B], start=True, stop=False)
            nc.tensor.matmul(sl, ulw[1][beta], h[:, B:2 * B], start=False, stop=False)
            nc.tensor.matmul(sl, wlw[beta], xs, start=False, stop=True)
        S = gates.tile([128, 192], FP32, tag="sig")
        nc.scalar.activation(S, pz[:, 0:192], AF.Sigmoid)
        TG = gates.tile([128, 64], FP32, tag="tg")
        nc.scalar.activation(TG, pz[:, 192:256], AF.Tanh)
        t1 = gates.tile([128, 64], FP32, tag="t1")
        nc.vector.tensor_mul(t1, S[:, 0:64], TG)            # i * g
        t2 = gates.tile([128, 64], FP32, tag="t2")
        nc.vector.tensor_mul(t2, S[:, 64:128], c_st)        # f * c
        nc.vector.tensor_add(c_st, t1, t2)                  # c'
        TC = gates.tile([128, 64], FP32, tag="tc")
        nc.scalar.activation(TC, c_st, AF.Tanh)
        nc.vector.tensor_mul(h, S[:, 128:192], TC)          # h = o * tanh(c)
        nc.vector.tensor_max(hmax, hmax, h)

    # ---------------- output ----------------
    p_out = pmm.tile([1, B], FP32, tag="mm", name="p_out")
    nc.tensor.matmul(p_out, wob[0], hmax[:, 0:B], start=True, stop=False)
    nc.tensor.matmul(p_out, wob[1], hmax[:, B:2 * B], start=False, stop=False)
    nc.tensor.matmul(p_out, bob, ones_r, start=False, stop=True)
    out_sb = scratch.tile([1, B], FP32)
    _elu(nc, scratch, out_sb, p_out, [1, B])
    # DRAM is linear, so view [B,1] as [1,B] and copy the row straight out of
    # partition 0. (Rearranging the SBUF side to [B,1] instead passes CoreSim
    # but generates a wrong transposing-DMA descriptor on hardware.)
    nc.sync.dma_start(out=d_out.rearrange("b one -> one b"), in_=out_sb)


# ------------------------------------------------------------------
# host-side entry point
# ------------------------------------------------------------------
_CACHE = {}


def _shard_inputs(inputs, T):
    """Split batch across cores; replicate weights."""
    batch_keys = ["action", "osc", "motion_state", "robot_state", "mu", "mean"]
    wkeys = ["Wm", "bm", "Wr", "br", "Wc", "bc", "Wor", "bor", "boi",
             "Wl", "bl", "Ul", "Wo", "bo"]
    in_maps = []
    for i in range(NCORES):
        s = slice(B * i, B * (i + 1))
        m = {}
        for k in batch_keys:
            v = np.ascontiguousarray(np.asarray(inputs[k], dtype=np.float32)[s])
            if k in ("action", "osc"):
                v = v[:, :T]
            m[k] = np.ascontiguousarray(v)
        for k in wkeys:
            m[k] = np.ascontiguousarray(np.asarray(inputs[k], dtype=np.float32))
        in_maps.append(m)
    return in_maps


def kernel(**inputs) -> np.ndarray:
    from concourse.bass_utils import run_bass_kernel_spmd

    T = int(np.asarray(inputs["action"]).shape[1])
    if T not in _CACHE:
        _CACHE[T] = build_nc(T)
    nc = _CACHE[T]
    in_maps = _shard_inputs(inputs, T)
    res = run_bass_kernel_spmd(nc, in_maps, list(range(NCORES)))
    out = np.concatenate([res.results[i]["out"] for i in range(NCORES)], axis=0)
    return out.astype(np.float32)


if __name__ == "__main__":
    nc = build_nc(64)
    print("built ok:", len(nc.m.functions[0].instructions) if hasattr(nc.m.functions[0], "instructions") else "n/a")


# revision 18
# speedup vs baseline: 1.1459x; 1.1459x over previous
"""Trainium2 Bass kernel for nn_Critic (LSTM critic over T=512 steps).

Sharding: pure data parallel. B=256 batch rows are split across 8 cores
(32 rows each); all weights are replicated. The sequential LSTM scan runs
locally per core.

Everything on-chip is kept "feature major" (hidden unit on the partition
axis, batch on the free axis) so the recurrence needs no transposes:
  h, c, hmax       : [128, 64]   col = 32*k + b   (unit u = 128*k + p)
  z (gates, PSUM)  : [128, 256]  blocks of 32 cols = [i0 i1 f0 f1 o0 o1 g0 g1]
  xT (input proj)  : [97, T*32]  col = 32*t + b, rows 0:32 act, 32:96 inp2,
                                 row 96 = ones (carries fused bias)

Per scan step: 24 accumulating matmuls (16 Ul tiles + 8 [Wl|bias] tiles),
then sigmoid/tanh on ACT and the cell update on DVE.

Reference quirks honored:
  * inp3 = elu(empty @ Woi + boi) = elu(boi) broadcast -> constant; its
    contribution inp3 @ Wl[96:160] is folded into the z bias.
  * osc_state and Woi (shape [0,64]) are unused.
  * only osc[..., :64] is ever read.
"""

import os
import sys

sys.path.insert(0, "/opt/trn_rl_repo")

from contextlib import ExitStack

import numpy as np

import concourse.bass as bass
import concourse.bacc as bacc
import concourse.mybir as mybir
import concourse.tile as tile
from concourse.masks import make_identity

FP32 = mybir.dt.float32
AF = mybir.ActivationFunctionType
ALU = mybir.AluOpType

# Problem dims
B_FULL, T_FULL, A = 256, 512, 32
DM, DR = 64, 128
U = 256                 # lstm units (== combine units)
OSC_HALF = 64
NCORES = 8
B = B_FULL // NCORES    # 32 batch rows per core
XROWS = A + OSC_HALF    # 96 feature rows of xT (plus a ones row)

# psum gate-block beta -> source 128-col chunk of [Ul | Wl | bias] matrices.
# z layout blocks: [i0 i1 f0 f1 o0 o1 g0 g1]; weight col order is [i f g o].
SRC_CHUNK = [0, 1, 2, 3, 6, 7, 4, 5]


def _elu(nc, pool, out_ap, y_ap, shape):
    """out = elu(y) = max(y, exp(min(y, 0)) - 1), exact.

    y_ap may live in PSUM or SBUF. 2 DVE ops + 1 ACT op.
    """
    m = pool.tile(shape, FP32, tag="elu_m")
    nc.vector.tensor_scalar_min(m, y_ap, 0.0)
    e = pool.tile(shape, FP32, tag="elu_e")
    nc.scalar.activation(e, m, AF.Exp)
    # out = (e + (-1)) max y
    nc.vector.scalar_tensor_tensor(out_ap, e, -1.0, y_ap, ALU.add, ALU.max)


def build_nc(T=T_FULL):
    """Build the SPMD Bass program for one core (batch shard of 32)."""
    # Bacc (not plain Bass): its compile() runs move_matmul_waits_to_ldweights
    # + generate_event_semaphores, which split multi-wait instructions that
    # walrus codegen rejects ("Too many sync wait commands").
    nc = bacc.Bacc("TRN2", target_bir_lowering=False, debug=False)

    # ---- external I/O (per-core shard shapes) ----
    d_action = nc.dram_tensor("action", [B, T, A], FP32, kind="ExternalInput").ap()
    d_osc = nc.dram_tensor("osc", [B, T, OSC_HALF], FP32, kind="ExternalInput").ap()
    d_motion = nc.dram_tensor("motion_state", [B, DM], FP32, kind="ExternalInput").ap()
    d_robot = nc.dram_tensor("robot_state", [B, DR], FP32, kind="ExternalInput").ap()
    d_mu = nc.dram_tensor("mu", [B, A], FP32, kind="ExternalInput").ap()
    d_mean = nc.dram_tensor("mean", [B, A], FP32, kind="ExternalInput").ap()
    d_Wm = nc.dram_tensor("Wm", [DM, U], FP32, kind="ExternalInput").ap()
    d_bm = nc.dram_tensor("bm", [U], FP32, kind="ExternalInput").ap()
    d_Wr = nc.dram_tensor("Wr", [DR, U], FP32, kind="ExternalInput").ap()
    d_br = nc.dram_tensor("br", [U], FP32, kind="ExternalInput").ap()
    d_Wc = nc.dram_tensor("Wc", [2 * U, U], FP32, kind="ExternalInput").ap()
    d_bc = nc.dram_tensor("bc", [U], FP32, kind="ExternalInput").ap()
    d_Wor = nc.dram_tensor("Wor", [OSC_HALF, OSC_HALF], FP32, kind="ExternalInput").ap()
    d_bor = nc.dram_tensor("bor", [OSC_HALF], FP32, kind="ExternalInput").ap()
    d_boi = nc.dram_tensor("boi", [OSC_HALF], FP32, kind="ExternalInput").ap()
    d_Wl = nc.dram_tensor("Wl", [A + 2 * OSC_HALF, 4 * U], FP32, kind="ExternalInput").ap()
    d_bl = nc.dram_tensor("bl", [4 * U], FP32, kind="ExternalInput").ap()
    d_Ul = nc.dram_tensor("Ul", [U, 4 * U], FP32, kind="ExternalInput").ap()
    d_Wo = nc.dram_tensor("Wo", [U, 1], FP32, kind="ExternalInput").ap()
    d_bo = nc.dram_tensor("bo", [1], FP32, kind="ExternalInput").ap()
    d_out = nc.dram_tensor("out", [B, 1], FP32, kind="ExternalOutput").ap()

    with tile.TileContext(nc) as tc, ExitStack() as ctx:
        _build_body(
            ctx, tc, T,
            d_action, d_osc, d_motion, d_robot, d_mu, d_mean,
            d_Wm, d_bm, d_Wr, d_br, d_Wc, d_bc, d_Wor, d_bor, d_boi,
            d_Wl, d_bl, d_Ul, d_Wo, d_bo, d_out,
        )
    nc.finalize()
    return nc


def _build_body(ctx, tc, T,
                d_action, d_osc, d_motion, d_robot, d_mu, d_mean,
                d_Wm, d_bm, d_Wr, d_br, d_Wc, d_bc, d_Wor, d_bor, d_boi,
                d_Wl, d_bl, d_Ul, d_Wo, d_bo, d_out):
    nc = tc.nc
    TCH = T // 128          # 128-step chunks per batch row

    consts = ctx.enter_context(tc.tile_pool(name="consts", bufs=1))
    weights = ctx.enter_context(tc.tile_pool(name="weights", bufs=1))
    state = ctx.enter_context(tc.tile_pool(name="state", bufs=1))
    stage = ctx.enter_context(tc.tile_pool(name="stage", bufs=3))
    ptrans = ctx.enter_context(tc.tile_pool(name="ptrans", bufs=2, space="PSUM"))
    pmm = ctx.enter_context(tc.tile_pool(name="pmm", bufs=2, space="PSUM"))
    scratch = ctx.enter_context(tc.tile_pool(name="scratch", bufs=3))

    ident = consts.tile([128, 128], FP32)
    make_identity(nc, ident)
    ones_r = consts.tile([1, B], FP32)
    nc.vector.memset(ones_r, 1.0)

    # ---------------- weights to SBUF ----------------
    # Ul tiles [128,128]: ulw[k][beta] = Ul[128k:128k+128, chunk SRC_CHUNK[beta]]
    ulw = [[weights.tile([128, 128], FP32, tag=f"ul_{k}_{b}", name=f"ul_{k}_{b}") for b in range(8)]
           for k in range(2)]
    for k in range(2):
        for beta in range(8):
            m = SRC_CHUNK[beta]
            nc.sync.dma_start(
                out=ulw[k][beta],
                in_=d_Ul[128 * k:128 * (k + 1), 128 * m:128 * (m + 1)])

    # Wl tiles [97,128]: rows 0:96 = Wl[0:96, chunk], row 96 = fused bias
    # xT row layout: 0:64 = inp2 features, 64:96 = action features, 96 = ones
    # (engine APs must start at partition 0/32/64/96; a 64-row write may not
    # start at partition 32, so inp2 goes first). Wl tile rows permuted to match.
    wlw = [weights.tile([XROWS + 1, 128], FP32, tag=f"wl_{b}", name=f"wl_{b}") for b in range(8)]
    for beta in range(8):
        m = SRC_CHUNK[beta]
        nc.sync.dma_start(out=wlw[beta][0:OSC_HALF, :],
                          in_=d_Wl[A:A + OSC_HALF, 128 * m:128 * (m + 1)])
        nc.sync.dma_start(out=wlw[beta][OSC_HALF:XROWS, :],
                          in_=d_Wl[0:A, 128 * m:128 * (m + 1)])

    # fused bias blEff = bl + elu(boi) @ Wl[96:160, :]
    boi_sb = scratch.tile([OSC_HALF, 1], FP32)
    nc.sync.dma_start(out=boi_sb, in_=d_boi.rearrange("(p one) -> p one", one=1))
    eboi = scratch.tile([OSC_HALF, 1], FP32)
    _elu(nc, scratch, eboi, boi_sb, [OSC_HALF, 1])
    wl_hi = scratch.tile([OSC_HALF, 4 * U], FP32)
    nc.sync.dma_start(out=wl_hi, in_=d_Wl[XROWS:XROWS + OSC_HALF, :])
    p_bl = pmm.tile([1, 4 * U], FP32, tag="p_bl", bufs=1)
    for half in range(2):
        nc.tensor.matmul(p_bl[:, 512 * half:512 * (half + 1)],
                         eboi, wl_hi[:, 512 * half:512 * (half + 1)],
                         start=True, stop=True)
    bl_sb = scratch.tile([1, 4 * U], FP32)
    nc.sync.dma_start(out=bl_sb, in_=d_bl.rearrange("(one n) -> one n", one=1))
    bleff = scratch.tile([1, 4 * U], FP32)
    nc.vector.tensor_add(bleff, p_bl, bl_sb)
    for beta in range(8):
        m = SRC_CHUNK[beta]
        nc.vector.tensor_copy(wlw[beta][XROWS:XROWS + 1, :],
                              bleff[:, 128 * m:128 * (m + 1)])

    # [Wor; bor] [65, 64]
    worb = weights.tile([OSC_HALF + 1, OSC_HALF], FP32)
    nc.sync.dma_start(out=worb[0:OSC_HALF, :], in_=d_Wor)
    nc.sync.dma_start(out=worb[OSC_HALF:OSC_HALF + 1, :],
                      in_=d_bor.rearrange("(one n) -> one n", one=1))

    # [Wm; bm] chunks [65, 128]
    wmb = [weights.tile([DM + 1, 128], FP32, tag=f"wm_{c}", name=f"wm_{c}") for c in range(2)]
    for c in range(2):
        nc.sync.dma_start(out=wmb[c][0:DM, :], in_=d_Wm[:, 128 * c:128 * (c + 1)])
        nc.sync.dma_start(out=wmb[c][DM:DM + 1, :],
                          in_=d_bm.rearrange("(one n) -> one n", one=1)[:, 128 * c:128 * (c + 1)])
    # Wr chunks [128,128] + br rows [1,128]
    wrb = [weights.tile([DR, 128], FP32, tag=f"wr_{c}", name=f"wr_{c}") for c in range(2)]
    brb = [weights.tile([1, 128], FP32, tag=f"br_{c}", name=f"br_{c}") for c in range(2)]
    for c in range(2):
        nc.sync.dma_start(out=wrb[c], in_=d_Wr[:, 128 * c:128 * (c + 1)])
        nc.sync.dma_start(out=brb[c],
                          in_=d_br.rearrange("(one n) -> one n", one=1)[:, 128 * c:128 * (c + 1)])
    # Wc chunks [128,128] (4 k-chunks x 2 m-chunks) + bc rows
    wcb = [[weights.tile([128, 128], FP32, tag=f"wc_{k}_{c}", name=f"wc_{k}_{c}") for c in range(2)]
           for k in range(4)]
    bcb = [weights.tile([1, 128], FP32, tag=f"bc_{c}", name=f"bc_{c}") for c in range(2)]
    for k in range(4):
        for c in range(2):
            nc.sync.dma_start(out=wcb[k][c],
                              in_=d_Wc[128 * k:128 * (k + 1), 128 * c:128 * (c + 1)])
    for c in range(2):
        nc.sync.dma_start(out=bcb[c],
                          in_=d_bc.rearrange("(one n) -> one n", one=1)[:, 128 * c:128 * (c + 1)])
    # Wo chunks [128,1], bo [1,1]
    wob = [weights.tile([128, 1], FP32, tag=f"wo_{c}", name=f"wo_{c}") for c in range(2)]
    for c in range(2):
        nc.sync.dma_start(out=wob[c], in_=d_Wo[128 * c:128 * (c + 1), :])
    bob = weights.tile([1, 1], FP32)
    nc.sync.dma_start(out=bob, in_=d_bo.rearrange("(one n) -> one n", one=1))

    # muT/meanT [32a, 32b] via PE transpose
    mu_sb = scratch.tile([B, A], FP32)
    mean_sb = scratch.tile([B, A], FP32)
    nc.sync.dma_start(out=mu_sb, in_=d_mu)
    nc.sync.dma_start(out=mean_sb, in_=d_mean)
    muT = consts.tile([A, B], FP32)
    meanT = consts.tile([A, B], FP32)
    for src, dst in ((mu_sb, muT), (mean_sb, meanT)):
        pt = ptrans.tile([A, B], FP32, tag="pt", name="pt_mu")
        nc.tensor.transpose(pt, src, ident[0:B, 0:B])
        nc.vector.tensor_copy(dst, pt)

    # ---------------- xT: [97, T*32] feature-major input projection ----------
    xT = state.tile([XROWS + 1, T * B], FP32)
    nc.vector.memset(xT[XROWS:XROWS + 1, :], 1.0)
    if os.environ.get("KERNEL_SKIP_PRE"):
        nc.vector.memset(xT[0:XROWS, :], 0.01)

    # action -> xT[0:32]: per (b, j): transpose [128t, 32] -> [32, 128t],
    # then out = psum * muT[:,b] + meanT[:,b] scattered to cols 32*t + b.
    PRE_B = 0 if os.environ.get("KERNEL_SKIP_PRE") else B
    for b in range(PRE_B):
        for j in range(TCH):
            a_tile = stage.tile([128, A], FP32, tag="a_in")
            nc.sync.dma_start(out=a_tile, in_=d_action[b, 128 * j:128 * (j + 1), :])
            pt = ptrans.tile([A, 128], FP32, tag="pt", name="pt_a")
            nc.tensor.transpose(pt, a_tile, ident)
            dst = xT[OSC_HALF:XROWS, :].rearrange("p (t b) -> p t b", b=B)[:, 128 * j:128 * (j + 1), b]
            nc.vector.tensor_scalar(dst, pt, muT[:, b:b + 1], meanT[:, b:b + 1],
                                    ALU.mult, ALU.add)

    # osc[...,:64] -> inp2 -> xT[32:96]: per b: 4 transposes into oscT_b
    # [65, 512], one matmul vs [Wor;bor], elu, scatter to cols 32*t + b.
    for b in range(PRE_B):
        oscT_b = stage.tile([OSC_HALF + 1, T], FP32, tag="oscT")
        nc.vector.memset(oscT_b[OSC_HALF:OSC_HALF + 1, :], 1.0)
        for j in range(TCH):
            o_tile = stage.tile([128, OSC_HALF], FP32, tag="o_in")
            nc.sync.dma_start(out=o_tile,
                              in_=d_osc[b, 128 * j:128 * (j + 1), :])
            pt = ptrans.tile([OSC_HALF, 128], FP32, tag="pt", name="pt_o")
            nc.tensor.transpose(pt, o_tile, ident)
            nc.vector.tensor_copy(oscT_b[0:OSC_HALF, 128 * j:128 * (j + 1)], pt)
        n_mm = (T + 511) // 512
        pw = pmm.tile([OSC_HALF, T], FP32, tag="mm", name="pw")
        for q in range(n_mm):
            w = min(512, T - 512 * q)
            nc.tensor.matmul(pw[:, 512 * q:512 * q + w], worb,
                             oscT_b[:, 512 * q:512 * q + w], start=True, stop=True)
        dst = xT[0:OSC_HALF, :].rearrange("p (t b) -> p t b", b=B)[:, :, b]
        _elu(nc, scratch, dst, pw, [OSC_HALF, T])

    # ---------------- h0 = c0 ----------------
    motT = scratch.tile([DM + 1, B], FP32)
    pt = ptrans.tile([DM, B], FP32, tag="pt", name="pt_mot")
    mot_sb = scratch.tile([B, DM], FP32)
    nc.sync.dma_start(out=mot_sb, in_=d_motion)
    nc.tensor.transpose(pt, mot_sb, ident[0:B, 0:B])
    nc.vector.tensor_copy(motT[0:DM, :], pt)
    nc.vector.memset(motT[DM:DM + 1, :], 1.0)

    robT = scratch.tile([DR, B], FP32)
    pt = ptrans.tile([DR, B], FP32, tag="pt", name="pt_rob")
    rob_sb = scratch.tile([B, DR], FP32)
    nc.sync.dma_start(out=rob_sb, in_=d_robot)
    nc.tensor.transpose(pt, rob_sb, ident[0:B, 0:B])
    nc.vector.tensor_copy(robT, pt)

    # ms = elu(motion @ Wm + bm): psum [128, 64] (col = 32*c + b)
    p_ms = pmm.tile([128, 2 * B], FP32, tag="mm", name="p_ms")
    for c in range(2):
        nc.tensor.matmul(p_ms[:, B * c:B * (c + 1)], wmb[c], motT,
                         start=True, stop=True)
    msT = scratch.tile([128, 2 * B], FP32, tag="msT")
    _elu(nc, scratch, msT, p_ms, [128, 2 * B])

    p_rs = pmm.tile([128, 2 * B], FP32, tag="mm", name="p_rs")
    for c in range(2):
        sl = p_rs[:, B * c:B * (c + 1)]
        nc.tensor.matmul(sl, wrb[c], robT, start=True, stop=False)
        nc.tensor.matmul(sl, brb[c], ones_r, start=False, stop=True)
    rsT = scratch.tile([128, 2 * B], FP32, tag="rsT")
    _elu(nc, scratch, rsT, p_rs, [128, 2 * B])

    p_st = pmm.tile([128, 2 * B], FP32, tag="mm", name="p_st")
    for c in range(2):
        sl = p_st[:, B * c:B * (c + 1)]
        nc.tensor.matmul(sl, wcb[0][c], msT[:, 0:B], start=True, stop=False)
        nc.tensor.matmul(sl, wcb[1][c], msT[:, B:2 * B], start=False, stop=False)
        nc.tensor.matmul(sl, wcb[2][c], rsT[:, 0:B], start=False, stop=False)
        nc.tensor.matmul(sl, wcb[3][c], rsT[:, B:2 * B], start=False, stop=False)
        nc.tensor.matmul(sl, bcb[c], ones_r, start=False, stop=True)

    h = state.tile([128, 2 * B], FP32)
    c_st = state.tile([128, 2 * B], FP32)
    hmax = state.tile([128, 2 * B], FP32)
    _elu(nc, scratch, h, p_st, [128, 2 * B])
    nc.vector.tensor_copy(c_st, h)
    nc.vector.memset(hmax, -1e30)

    # ---------------- the scan ----------------
    gates = ctx.enter_context(tc.tile_pool(name="gates", bufs=2))
    pz_pool = ctx.enter_context(tc.tile_pool(name="pz", bufs=2, space="PSUM"))
    BETA_ORDER = [6, 7, 0, 1, 2, 3, 4, 5]   # g first, o last
    T_SCAN = 0 if os.environ.get("KERNEL_SKIP_SCAN") else T
    for t in range(T_SCAN):
        pz = pz_pool.tile([128, 256], FP32)
        xs = xT[:, B * t:B * (t + 1)]
        for beta in BETA_ORDER:
            sl = pz[:, 32 * beta:32 * (beta + 1)]
            nc.tensor.matmul(sl, ulw[0][beta], h[:, 0:B], start=True, stop=False)
            nc.tensor.matmul(sl, ulw[1][beta], h[:, B:2 * B], start=False, stop=False)
            nc.tensor.matmul(sl, wlw[beta], xs, start=False, stop=True)
        S = gates.tile([128, 192], FP32, tag="sig")
        nc.scalar.activation(S, pz[:, 0:192], AF.Sigmoid)
        TG = gates.tile([128, 64], FP32, tag="tg")
        nc.scalar.activation(TG, pz[:, 192:256], AF.Tanh)
        t1 = gates.tile([128, 64], FP32, tag="t1")
        nc.vector.tensor_mul(t1, S[:, 0:64], TG)            # i * g
        t2 = gates.tile([128, 64], FP32, tag="t2")
        nc.vector.tensor_mul(t2, S[:, 64:128], c_st)        # f * c
        nc.vector.tensor_add(c_st, t1, t2)                  # c'
        TC = gates.tile([128, 64], FP32, tag="tc")
        nc.scalar.activation(TC, c_st, AF.Tanh)
        nc.vector.tensor_mul(h, S[:, 128:192], TC)          # h = o * tanh(c)
        nc.vector.tensor_max(hmax, hmax, h)

    # ---------------- output ----------------
    p_out = pmm.tile([1, B], FP32, tag="mm", name="p_out")
    nc.tensor.matmul(p_out, wob[0], hmax[:, 0:B], start=True, stop=False)
    nc.tensor.matmul(p_out, wob[1], hmax[:, B:2 * B], start=False, stop=False)
    nc.tensor.matmul(p_out, bob, ones_r, start=False, stop=True)
    out_sb = scratch.tile([1, B], FP32)
    _elu(nc, scratch, out_sb, p_out, [1, B])
    # DRAM is linear, so view [B,1] as [1,B] and copy the row straight out of
    # partition 0. (Rearranging the SBUF side to [B,1] instead passes CoreSim
    # but generates a wrong transposing-DMA descriptor on hardware.)
    nc.sync.dma_start(out=d_out.rearrange("b one -> one b"), in_=out_sb)


# ------------------------------------------------------------------
# host-side entry point
# ------------------------------------------------------------------
_CACHE = {}


def _shard_inputs(inputs, T):
    """Split batch across cores; replicate weights."""
    batch_keys = ["action", "osc", "motion_state", "robot_state", "mu", "mean"]
    wkeys = ["Wm", "bm", "Wr", "br", "Wc", "bc", "Wor", "bor", "boi",
             "Wl", "bl", "Ul", "Wo", "bo"]
    in_maps = []
    for i in range(NCORES):
        s = slice(B * i, B * (i + 1))
        m = {}
        for k in batch_keys:
            v = np.asarray(inputs[k], dtype=np.float32)[s]
            if k == "action":
                v = v[:, :T]
            elif k == "osc":
                # only the first half of the osc features is ever read
                v = v[:, :T, :OSC_HALF]
            m[k] = np.ascontiguousarray(v)
        for k in wkeys:
            m[k] = np.ascontiguousarray(np.asarray(inputs[k], dtype=np.float32))
        in_maps.append(m)
    return in_maps


def kernel(**inputs) -> np.ndarray:
    from concourse.bass_utils import run_bass_kernel_spmd

    T = int(np.asarray(inputs["action"]).shape[1])
    if T not in _CACHE:
        _CACHE[T] = build_nc(T)
    nc = _CACHE[T]
    in_maps = _shard_inputs(inputs, T)
    res = run_bass_kernel_spmd(nc, in_maps, list(range(NCORES)))
    out = np.concatenate([res.results[i]["out"] for i in range(NCORES)], axis=0)
    return out.astype(np.float32)


if __name__ == "__main__":
    nc = build_nc(64)
    print("built ok:", len(nc.m.functions[0].instructions) if hasattr(nc.m.functions[0], "instructions") else "n/a")
